# revision 1
# baseline (speedup 1.0000x reference)
# Multi-head attention (B=2, S=2048, D=1024, H=16, head_dim=64) with bool mask,
# sharded across 8 TRN2 NeuronCores: core c -> batch c//4, heads 4*(c%4)..4*(c%4)+3.
#
# Per-core device kernel (scores computed transposed: scoresT[k, q]):
#   scoresT = K @ Q^T                (PE bf16, lhsT = K^T chunk, rhs = Q^T)
#   attnT   = exp(scoresT/8) * (1-m)T (ACT exp with scale=1/8 -> bf16, DVE mult)
#   outT'   = [V | 1]^T @ attnT      (PE bf16; row 64 = softmax denominator Z)
#   out     = transpose(outT') / Z   (PE transpose + batched DVE normalize)
#
# Host side (inside kernel()): slice per-core shards, pre-transpose Q/K per head
# ([64, S] head-dim-major, bf16), pre-transpose the inverted mask to bf16,
# reassemble the 8 per-core bf16 outputs into the full f32 [B, S, D] output.

import sys

import numpy as np

for _p in ("/opt/trn_rl_repo",):
    if _p not in sys.path:
        sys.path.insert(0, _p)

import ml_dtypes

import concourse.bass as bass  # noqa: F401  (engine types reachable via nc)
import concourse.tile as tile
from concourse import bacc, mybir
from concourse.bass_utils import run_bass_kernel_spmd
from concourse.masks import make_identity

F32 = mybir.dt.float32
F32R = mybir.dt.float32r
BF16 = mybir.dt.bfloat16

S = 2048          # sequence length
HD = 64           # head dim
HPC = 4           # heads per core
NCORES = 8
B = 2
H = 16
D = H * HD


def build_program(s=S, act_dtype=BF16, qk_dtype=BF16, n_psS=2, reps=1):
    """Build the single-core SPMD program. Returns the compiled Bacc object.

    reps>1 emits the whole body (loads+compute+stores) that many times in one
    NEFF — used to measure device time by wall-clock differencing."""
    nc = bacc.Bacc()

    KS = s // 128            # number of k strips
    QG = 1024 if s >= 1024 else s   # q group width (ACT/DVE instruction width)
    NQG = s // QG            # q groups
    NQC = max(QG // 512, 1)  # 512-wide matmul chunks per q group
    QC = min(512, QG)        # matmul chunk width
    JT = QG // 128           # out-transpose chunks per q group

    qkT_d = nc.declare_dram_parameter("qkT", [2, HPC * HD, s], qk_dtype, isOutput=False)
    v_d = nc.declare_dram_parameter("v", [s, HPC * HD], BF16, isOutput=False)
    nmT_d = nc.declare_dram_parameter("nmT", [s, s], BF16, isOutput=False)
    out_d = nc.declare_dram_parameter("out", [s, HPC * HD], BF16, isOutput=True)

    # DRAM views with the k/q axis split into strips of 128 partitions
    nm_view = nmT_d[:].rearrange("(ks p) q -> p ks q", p=128)
    v_view = v_d[:].rearrange("(ks p) c -> p ks c", p=128)
    out_view = out_d[:].rearrange("(sq p) c -> p sq c", p=128)

    with tile.TileContext(nc) as tc:
        with (
            tc.tile_pool(name="const", bufs=1) as const,
            tc.tile_pool(name="wq", bufs=1) as wq,
            tc.tile_pool(name="vstg", bufs=1) as vstg,
            tc.tile_pool(name="attn", bufs=20) as apool,
            tc.tile_pool(name="fin", bufs=2) as fpool,
            tc.tile_pool(name="stat", bufs=4) as spool,
            tc.tile_pool(name="oasm", bufs=1) as opool,
            tc.tile_pool(name="psS", bufs=n_psS, space="PSUM") as psS_pool,
            # psO (AV accumulator, [65,QG]=2 banks) and pn (out-transpose
            # target, [128,JT,128]=2 banks) share one tag with bufs=2: the
            # two slots alternate psO/pn roles, so AV of group g only waits
            # for the finalize reads of group g-2 (1.5 groups of slack).
            tc.tile_pool(name="psF", bufs=2, space="PSUM") as psF_pool,
        ):
            ident = const.tile([128, 128], F32)
            make_identity(nc, ident)

            # Preload the exp table (emitted before any real exp; runs while
            # the first DMAs stream).
            warm = const.tile([128, 1], F32)
            nc.vector.memset(warm, 0.0)
            nc.scalar.activation(warm, warm, mybir.ActivationFunctionType.Exp)

            # Warm the PE HAM clock gate while input DMAs run: ~3us of dummy
            # matmuls (transpose-mode doesn't count as PE-busy for HAM) so
            # the first real QKs run at 2.4GHz.
            zb = const.tile([128, 128], BF16)
            nc.vector.memset(zb, 0.0)
            for _ in range(24):
                wmm = psS_pool.tile([128, QG], F32, tag="psS")
                nc.tensor.matmul(
                    wmm[:, :128], lhsT=zb[0:64, :], rhs=zb[0:64, :],
                    start=True, stop=True,
                )

            def qk_src(pair):
                return qkT_d[:, 128 * pair:128 * pair + 128, :].rearrange(
                    "t p s -> p t s"
                )

            def emit_body():
                # Q^T / K^T head pairs: [128, s] (head 2p on partitions 0-63,
                # head 2p+1 on partitions 64-127). The first pair's q and k
                # halves ride different HWDGE queues in parallel; everything
                # else is emitted in the order compute consumes it.
                qks = []
                for pair in range(HPC // 2):
                    qk = wq.tile([128, 2, s], qk_dtype, tag=f"qkT{pair}")
                    qks.append(qk)
                v_sb = vstg.tile([128, KS, HPC * HD], BF16)
                nm_sb = wq.tile([128, KS, s], BF16, tag="nm")
                KH = KS // 2
                nc.scalar.dma_start(out=qks[0][:, 0, :], in_=qk_src(0)[:, 0, :])
                nc.sync.dma_start(out=qks[0][:, 1, :], in_=qk_src(0)[:, 1, :])
                nc.sync.dma_start(out=v_sb[:, :KH], in_=v_view[:, :KH])
                nc.sync.dma_start(out=v_sb[:, KH:], in_=v_view[:, KH:])
                for pair in range(1, HPC // 2):
                    nc.scalar.dma_start(out=qks[pair], in_=qk_src(pair))
                for ks in range(KS):
                    nc.sync.dma_start(out=nm_sb[:, ks, :], in_=nm_view[:, ks, :])

                # V' = [V | 1] per head, bf16; cast in halves so early AVs
                # only wait on the first half of the V DMA.
                vps = []
                for h in range(HPC):
                    vp = wq.tile([128, KS, HD + 1], BF16, tag=f"vp{h}")
                    vps.append(vp)
                for half in range(2):
                    ksl = slice(half * KH, KH + half * KH)
                    for h in range(HPC):
                        nc.vector.tensor_copy(
                            out=vps[h][:, ksl, 0:HD],
                            in_=v_sb[:, ksl, h * HD:(h + 1) * HD],
                        )
                        nc.vector.memset(vps[h][:, ksl, HD:HD + 1], 1.0)

                out_asm = opool.tile([128, KS, HPC * HD], BF16)

                # Emission state threading three overlapped group pipelines:
                #   carry - group awaiting last AV (stop=True) + psO->oT copy
                #   fin   - group awaiting its transpose+normalize steps
                fin = {"pend": None, "idx": 0, "pn": None}
                N_FIN = JT + 1  # JT transposes + one batched normalize step

                def finalize_step():
                    """One finalize chunk of a finished q-group: steps
                    0..JT-1 transpose [65,128] pieces into pn; step JT does
                    one strided reciprocal over the JT Z values and two
                    broadcast multiplies (batched - avoids per-strip
                    sequencer overhead)."""
                    h, qg, oT = fin["pend"]
                    if fin["idx"] >= N_FIN:
                        return
                    j = fin["idx"]
                    fin["idx"] += 1
                    if j == 0:
                        pn_t = psF_pool.tile([128, JT, 128], F32, tag="fin")
                        fin["pn"] = pn_t
                    pn = fin["pn"]
                    if j < JT:
                        nc.tensor.transpose(
                            pn[:, j, :HD + 1],
                            oT[:, j * 128:(j + 1) * 128],
                            ident[:HD + 1, :HD + 1],
                        )
                        return
                    rec8 = spool.tile([128, JT], F32)
                    nc.vector.reciprocal(rec8, pn[:, :, HD])
                    half = (JT + 1) // 2
                    for lo in range(0, JT, half):
                        hi = min(lo + half, JT)
                        sq0 = qg * JT + lo
                        nc.vector.tensor_mul(
                            out_asm[:, sq0:sq0 + hi - lo, h * HD:(h + 1) * HD],
                            pn[:, lo:hi, 0:HD],
                            rec8[:, lo:hi].to_broadcast([128, hi - lo, HD]),
                        )
                        if h == HPC - 1:
                            eng = nc.sync if lo == 0 else nc.scalar
                            eng.dma_start(
                                out=out_view[:, sq0:sq0 + hi - lo, :],
                                in_=out_asm[:, sq0:sq0 + hi - lo, :],
                            )

                def emit_carry(carry):
                    """Last AV (stop=True) + psO->SBUF copy of a group."""
                    ch, cqg, cpsO, cat = carry
                    for qc in range(NQC):
                        nc.tensor.matmul(
                            cpsO[:, qc * QC:(qc + 1) * QC],
                            lhsT=vps[ch][:, KS - 1, :],
                            rhs=cat[:, qc * QC:(qc + 1) * QC],
                            start=(KS == 1),
                            stop=True,
                        )
                    oT = fpool.tile([HD + 1, QG], F32, tag="oT")
                    nc.vector.tensor_copy(oT, cpsO)
                    # flush unfinished finalize steps of the older group
                    while fin["pend"] is not None and fin["idx"] < N_FIN:
                        finalize_step()
                    fin["pend"] = (ch, cqg, oT)
                    fin["idx"] = 0

                carry = None
                groups = [(h, qg) for h in range(HPC) for qg in range(NQG)]
                for h, qg in groups:
                    base = 64 * (h % 2)
                    qt_r = qks[h // 2][:, 0, :]
                    kt_r = qks[h // 2][:, 1, :]
                    q0 = qg * QG
                    psO = None
                    at_prev = None
                    for ks in range(KS):
                        # AV one strip behind QK, emitted BEFORE this strip's
                        # QK so it isn't queued behind QK's psum-slot wait.
                        if at_prev is not None:
                            if psO is None:
                                psO = psF_pool.tile(
                                    [HD + 1, QG], F32, tag="fin"
                                )
                            for qc in range(NQC):
                                nc.tensor.matmul(
                                    psO[:, qc * QC:(qc + 1) * QC],
                                    lhsT=vps[h][:, ks - 1, :],
                                    rhs=at_prev[:, qc * QC:(qc + 1) * QC],
                                    start=(ks == 1),
                                    stop=False,
                                )
                        # Transpose+normalize of an older group, interleaved
                        # so it never stalls the PE pipeline.
                        if fin["pend"] is not None and ks >= 1:
                            finalize_step()
                        psS = psS_pool.tile([128, QG], F32)
                        for qc in range(NQC):
                            nc.tensor.matmul(
                                psS[:, qc * QC:(qc + 1) * QC],
                                lhsT=kt_r[base:base + HD, ks * 128:(ks + 1) * 128],
                                rhs=qt_r[base:base + HD,
                                         q0 + qc * QC:q0 + (qc + 1) * QC],
                                start=True,
                                stop=True,
                            )
                        if ks == 0 and carry is not None:
                            emit_carry(carry)
                            carry = None
                        at = apool.tile([128, QG], act_dtype, tag="at")
                        nc.scalar.activation(
                            at, psS, mybir.ActivationFunctionType.Exp,
                            scale=0.125,
                        )
                        nc.vector.tensor_mul(at, at, nm_sb[:, ks, q0:q0 + QG])
                        at_prev = at
                    carry = (h, qg, psO, at_prev)
                emit_carry(carry)
                while fin["idx"] < N_FIN:
                    finalize_step()

            for _ in range(reps):
                emit_body()
    nc.compile()
    return nc


_CACHE = {}


def _get_nc():
    if "nc" not in _CACHE:
        _CACHE["nc"] = build_program()
    return _CACHE["nc"]


def make_in_maps(q, k, v, mask, s=S):
    """Shard full inputs into 8 per-core input maps (host-side layout prep)."""
    q = np.asarray(q, dtype=np.float32)
    k = np.asarray(k, dtype=np.float32)
    v = np.asarray(v, dtype=np.float32)
    mask = np.asarray(mask)
    nh = q.shape[-1] // HD
    in_maps = []
    for c in range(NCORES):
        b, g = divmod(c, NCORES // B)
        h0 = HPC * g
        qs = q[b].reshape(s, nh, HD)[:, h0:h0 + HPC, :]      # [s, HPC, 64]
        ks_ = k[b].reshape(s, nh, HD)[:, h0:h0 + HPC, :]
        qkT = np.empty((2, HPC * HD, s), ml_dtypes.bfloat16)
        qkT[0] = qs.transpose(1, 2, 0).reshape(HPC * HD, s)
        qkT[1] = ks_.transpose(1, 2, 0).reshape(HPC * HD, s)
        vc = np.ascontiguousarray(v[b, :, h0 * HD:(h0 + HPC) * HD]).astype(
            ml_dtypes.bfloat16
        )
        nmT = np.ascontiguousarray((~mask[b]).T).astype(ml_dtypes.bfloat16)
        in_maps.append({"qkT": qkT, "v": vc, "nmT": nmT})
    return in_maps


def assemble_out(results, s=S, d=D):
    out = np.empty((B, s, d), np.float32)
    for c in range(NCORES):
        b, g = divmod(c, NCORES // B)
        out[b, :, g * HPC * HD:(g + 1) * HPC * HD] = results[c]["out"]
    return out


def kernel(q, k, v, mask):
    nc = _get_nc()
    in_maps = make_in_maps(q, k, v, mask)
    res = run_bass_kernel_spmd(nc, in_maps, list(range(NCORES))).results
    return assemble_out(res)



# revision 19
# speedup vs baseline: 1.2048x; 1.2048x over previous
# Multi-head attention (B=2, S=2048, D=1024, H=16, head_dim=64) with bool mask,
# sharded across 8 TRN2 NeuronCores: core c -> batch c//4, heads 4*(c%4)..4*(c%4)+3.
#
# Per-core device kernel (scores computed transposed: scoresT[k, q]):
#   scoresT = K @ Q^T              (PE bf16, per 128-k strip, into PSUM)
#   at      = exp(scoresT/8)*nmT   split between two engines:
#              - ACT strips: ACT exp (scale=1/8) -> fp16, then DVE/Pool mask mul
#              - DVE strips: custom DVE op1 p=poly4(x)~e^(x/64) -> fp16,
#                custom DVE op2 at=(p^2)^2)^2*mask (mask fused; m^8=m)
#   psO     = [V|1]^T @ at         (PE fp16 accumulate over strips; row 64 = Z)
#   evac    = copy psO -> SBUF bf16 (DVE), DMA out (numerator+Z, unnormalized)
# Host side: shard inputs, pre-transpose Q/K per head (bf16), build V'=[V|1]
# (fp16), inverted-transposed mask (fp16); after run: out = (num/Z)^T merge.
#
# The custom DVE ops offload ~1/6 of the exp work from the ACT engine (the
# baseline bottleneck at ~133us busy) to the otherwise-idle DVE ALU pipeline,
# balancing PE/ACT/DVE at ~110us each.

import sys

import numpy as np

for _p in ("/opt/trn_rl_repo",):
    if _p not in sys.path:
        sys.path.insert(0, _p)

import ml_dtypes

import concourse.bass as bass  # noqa: F401  (engine types reachable via nc)
import concourse.tile as tile
from concourse import bacc, mybir
from concourse.bass_utils import run_bass_kernel_spmd

F32 = mybir.dt.float32
BF16 = mybir.dt.bfloat16
FP16 = mybir.dt.float16

S = 2048          # sequence length
HD = 64           # head dim
HPC = 4           # heads per core
NCORES = 8
B = 2
H = 16
D = H * HD

# e^(x/64) ~= 1 + c1 x + c2 x^2 + c3 x^3 + c4 x^4 on |x| <= 52 (~6.5 sigma of
# the N(0,64) score distribution); rel err 2.5e-4, x8 through the squarings
# -> ~2e-3 weight noise on DVE strips (output impact ~0.1%).
POLY_C = (1.5610800e-02, 1.2226440e-04, 6.6047774e-07, 2.4435055e-09)

_DVE_OPS = {}


def _register_dve_ops():
    """Register the two custom DVE ops (idempotent). Uses the documented
    extension point (dve_ops.OPS) at runtime since the repo is read-only."""
    if _DVE_OPS:
        return _DVE_OPS
    from concourse import dve_ops
    from concourse.dve_spec import (
        C0,
        C1,
        C2,
        C3,
        One,
        Spec,
        Src0,
        Src1,
        _has_src1,
        _spill_c3_to_src1,
        lower,
    )
    from concourse.dve_uop import DveOpSpec

    def _mk(name, body, reference, subdim=False):
        existing = {op.name: op for op in dve_ops.OPS}
        if name in existing:
            _DVE_OPS[name] = existing[name]
            return existing[name]
        spec = Spec(body=body, reference=reference)
        shas = {}
        for ver in ("v3", "v4"):
            try:
                uops = lower(spec, ver=ver)
                probe = DveOpSpec(
                    name=name, opcode=31, uops=uops, rd1_en=_has_src1(spec)
                )
                shas[ver] = probe.sha(ver)
            except Exception:
                pass
        op = dve_ops.DveOp(name, spec, subdim=subdim, uops_sha=shas)
        dve_ops.OPS.append(op)
        dve_ops.CUSTOM_DVE_SPECS[name] = spec
        dve_ops._SUB_OPCODE_FOR_NAME[name] = dve_ops._CUSTOM_DVE_ROW_BASE + (
            len(dve_ops.OPS) - 1
        )
        assert dve_ops._SUB_OPCODE_FOR_NAME[name] < 0x20
        _DVE_OPS[name] = op
        return op

    # op1: p = 1 + x(c1 + x(c2 + x(c3 + x c4)))   [8 ALU stages]
    # binding: s0=c4, s1=c3, imm2=c2, in1=[P,1] c1 (spilled C3)
    body1 = ((((Src0 * C0 + C1) * Src0 + C2) * Src0 + C3) * Src0) + One
    body1 = _spill_c3_to_src1(body1)

    def ref1(in0, in1, s0, s1, imm2):
        x = np.asarray(in0, np.float32)
        c1v = np.asarray(in1, np.float32)
        return ((((x * s0 + s1) * x + imm2) * x + c1v) * x) + np.float32(1.0)

    _mk("ANT_EXP64_P4", body1, ref1)

    # op2: at = (((p^2)^2)^2) * mask   [4 ALU stages]
    body2 = (Src0 * Src0)
    body2 = body2 * body2
    body2 = body2 * body2
    body2 = body2 * Src1

    def ref2(in0, in1, s0, s1, imm2):
        p = np.asarray(in0, np.float32)
        p8 = ((p * p) ** 2) ** 2
        return p8 * np.asarray(in1, np.float32)

    _mk("ANT_SQ3_MUL", body2, ref2)
    return _DVE_OPS


# Strips (by ks index) whose exp runs on the DVE via the custom ops, per
# group; and ACT strips whose mask-mul runs on Pool instead of DVE.
DVE_KS = (2, 6, 10)             # 3 per group x 8 groups = 24 DVE strips
POOL_MASK_KS = (0, 3, 7, 11)    # mask muls moved off DVE to the idle Pool


def build_program(s=S, reps=1, dve_ks=DVE_KS, pool_mask_ks=POOL_MASK_KS,
                  n_psS=3, n_psO=1):
    """Build the single-core SPMD program. Returns the compiled Bacc object."""
    ops = _register_dve_ops()
    op_poly = ops["ANT_EXP64_P4"]
    op_sq3 = ops["ANT_SQ3_MUL"]

    nc = bacc.Bacc()

    KS = s // 128            # number of k strips
    QG = 1024 if s >= 1024 else s   # q group width
    NQG = s // QG            # q groups
    NQC = max(QG // 512, 1)  # 512-wide matmul chunks per q group
    QC = min(512, QG)        # matmul chunk width
    NG = HPC * NQG           # groups per core

    c4, c3, c2, c1 = POLY_C[3], POLY_C[2], POLY_C[1], POLY_C[0]

    qkT_d = nc.declare_dram_parameter("qkT", [2, HPC * HD, s], BF16, isOutput=False)
    vp_d = nc.declare_dram_parameter("vp", [128, KS, HPC, HD + 1], FP16, isOutput=False)
    nmT_d = nc.declare_dram_parameter("nmT", [s, s], FP16, isOutput=False)
    out_d = nc.declare_dram_parameter("out", [NG, HD + 1, QG], BF16, isOutput=True)

    nm_view = nmT_d[:].rearrange("(ks p) q -> p ks q", p=128)

    with tile.TileContext(nc) as tc:
        with (
            tc.tile_pool(name="const", bufs=1) as const,
            tc.tile_pool(name="wq", bufs=1) as wq,
            tc.tile_pool(name="attn", bufs=20) as apool,
            tc.tile_pool(name="ppool", bufs=3) as ppool,
            tc.tile_pool(name="evac", bufs=2) as epool,
            tc.tile_pool(name="psS", bufs=n_psS, space="PSUM") as psS_pool,
            tc.tile_pool(name="psO", bufs=n_psO, space="PSUM") as psO_pool,
        ):
            # [P,1] scalar for the spilled 4th poly coefficient.
            c1t = const.tile([128, 1], F32)
            nc.vector.memset(c1t, c1)

            # Preload the exp table while the first DMAs stream.
            warm = const.tile([128, 1], F32)
            nc.vector.memset(warm, 0.0)
            nc.scalar.activation(warm, warm, mybir.ActivationFunctionType.Exp)

            # Warm the PE HAM clock gate: ~3us of dummy matmuls so the first
            # real QKs run at full frequency.
            zb = const.tile([128, 128], BF16)
            nc.vector.memset(zb, 0.0)
            for _ in range(24):
                wmm = psS_pool.tile([128, QG], F32, tag="psS")
                nc.tensor.matmul(
                    wmm[:, :128], lhsT=zb[0:64, :], rhs=zb[0:64, :],
                    start=True, stop=True,
                )

            def emit_body():
                # Input staging tiles
                qks = []
                for pair in range(HPC // 2):
                    qk = wq.tile([128, 2, s], BF16, tag=f"qkT{pair}")
                    qks.append(qk)
                vp_sb = wq.tile([128, KS, HPC, HD + 1], FP16, tag="vp")
                nm_sb = wq.tile([128, KS * s], FP16, tag="nm")

                def qk_src(pair):
                    return qkT_d[:, 128 * pair:128 * pair + 128, :].rearrange(
                        "t p s -> p t s"
                    )

                # DMA schedule: sync (SP HWDGE) carries the critical-path
                # early inputs; pool (SWDGE) carries late inputs + outputs.
                KH = KS // 2
                # k half of pair0 (cols 0:s) and q half cols of group 0.
                # Priority order for the serialized DMA-engine resource:
                # first QK operands for strips 0-3, then mask strips as the
                # consumer stream needs them, V' halves and the later q/k
                # windows just in time, second head pair and the qg1 mask
                # halves last.
                HQ = QG // 2
                nc.sync.dma_start(out=qks[0][:, 1, 0:HQ], in_=qk_src(0)[:, 1, 0:HQ])
                nc.sync.dma_start(out=qks[0][:, 0, 0:QG], in_=qk_src(0)[:, 0, 0:QG])
                nc.sync.dma_start(
                    out=qks[0][:, 1, HQ:QG], in_=qk_src(0)[:, 1, HQ:QG]
                )
                nm_dst = lambda ks, a, b: nm_sb[:, ks * s + a:ks * s + b]
                nc.sync.dma_start(out=nm_dst(0, 0, QG), in_=nm_view[:, 0, 0:QG])
                nc.sync.dma_start(out=nm_dst(1, 0, QG), in_=nm_view[:, 1, 0:QG])
                nc.sync.dma_start(out=vp_sb[:, :KH], in_=vp_d[:, :KH])
                nc.sync.dma_start(out=nm_dst(2, 0, QG), in_=nm_view[:, 2, 0:QG])
                nc.sync.dma_start(out=nm_dst(3, 0, QG), in_=nm_view[:, 3, 0:QG])
                if s > QG:
                    nc.sync.dma_start(
                        out=qks[0][:, 1, QG:s], in_=qk_src(0)[:, 1, QG:s]
                    )
                nc.sync.dma_start(out=vp_sb[:, KH:], in_=vp_d[:, KH:])
                for ks in range(4, KS):
                    nc.sync.dma_start(
                        out=nm_dst(ks, 0, QG), in_=nm_view[:, ks, 0:QG]
                    )
                    if ks == 8 and s > QG:
                        nc.sync.dma_start(
                            out=qks[0][:, 0, QG:s], in_=qk_src(0)[:, 0, QG:s]
                        )
                if s > QG:
                    # qg1 mask halves are consumed from slot 16 on; the
                    # second head pair only from slot 64 -> pair goes last
                    for ks in range(KS):
                        nc.sync.dma_start(
                            out=nm_dst(ks, QG, s), in_=nm_view[:, ks, QG:s]
                        )
                for pair in range(1, HPC // 2):
                    nc.sync.dma_start(out=qks[pair], in_=qk_src(pair))

                groups = [(h, qg) for h in range(HPC) for qg in range(NQG)]
                NT = NG * KS  # total strips

                # per-group / scheduler state
                psO = {}            # g -> psum tile
                n_chunks = {}       # g -> AV chunks emitted
                av_q = []           # (min_slot, g, h, qg, ks, at, qc)
                dve_q = []          # (min_slot, kind, payload) for DVE stream
                AV_BUDGET = 3       # AV matmul chunks per slot (PE smoothing;
                                    # 3 > steady-state 2 so backlogs drain)

                def emit_av_chunk(g, h, qg, ks, at, qc):
                    if n_chunks.get(g, 0) == 0:
                        psO[g] = psO_pool.tile(
                            [HD + 1, QG], F32, tag="psO", name=f"psO{g}"
                        )
                    n_chunks[g] = n_chunks.get(g, 0) + 1
                    first = n_chunks[g] <= NQC
                    last = n_chunks[g] > (KS - 1) * NQC
                    nc.tensor.matmul(
                        psO[g][:, qc * QC:(qc + 1) * QC],
                        lhsT=vp_sb[:, ks, h, :],
                        rhs=at[:, qc * QC:(qc + 1) * QC],
                        start=first,
                        stop=last,
                    )
                    return n_chunks[g] == KS * NQC

                def emit_evac(g):
                    ev = epool.tile([HD + 1, QG], BF16, tag="ev", name=f"ev{g}")
                    hq = QG // 2
                    nc.vector.tensor_copy(ev[:, 0:hq], psO[g][:, 0:hq])
                    nc.vector.tensor_copy(ev[:, hq:QG], psO[g][:, hq:QG])
                    nc.sync.dma_start(out=out_d[g], in_=ev[:])

                for t in range(NT + 8):
                    # 1) QK strip -> psS (its psS-slot dep resolves earliest)
                    if t < NT:
                        g, ks = divmod(t, KS)
                        h, qg = groups[g]
                        q0 = qg * QG
                        base = 64 * (h % 2)
                        qt_r = qks[h // 2][:, 0, :]
                        kt_r = qks[h // 2][:, 1, :]
                        is_dve = ks in dve_ks

                        psS = psS_pool.tile([128, QG], F32, tag="psS")
                        for qc in range(NQC):
                            nc.tensor.matmul(
                                psS[:, qc * QC:(qc + 1) * QC],
                                lhsT=kt_r[base:base + HD,
                                          ks * 128:(ks + 1) * 128],
                                rhs=qt_r[base:base + HD,
                                         q0 + qc * QC:q0 + (qc + 1) * QC],
                                start=True,
                                stop=True,
                            )

                    # 2) deferred DVE-stream work due now: these became ready
                    # before this slot's poly (DVE wait queues are FIFO, so
                    # emission order must match readiness order)
                    while dve_q and dve_q[0][0] <= t:
                        _, kind, pl = dve_q.pop(0)
                        if kind == "mul":
                            at_, nm_ = pl
                            nc.vector.tensor_mul(at_, at_, nm_)
                        elif kind == "sq3":
                            at_, p_, nm_ = pl
                            nc.vector._custom_dve(
                                op_sq3, out=at_[:], in0=p_[:], in1=nm_,
                            )
                        else:  # evac
                            emit_evac(pl)

                    if t < NT:
                        nm_ap = nm_sb[:, ks * s + q0:ks * s + q0 + QG]
                        at = apool.tile([128, QG], FP16, tag="at")
                        if is_dve:
                            p = ppool.tile([128, QG], FP16, tag="p")
                            nc.vector._custom_dve(
                                op_poly, out=p[:], in0=psS[:], in1=c1t[:],
                                s0=c4, s1=c3, imm2=c2,
                            )
                            dve_q.append((t + 2, "sq3", (at, p, nm_ap)))
                            delay = 5
                        else:
                            nc.scalar.activation(
                                at, psS, mybir.ActivationFunctionType.Exp,
                                scale=0.125,
                            )
                            if ks in pool_mask_ks:
                                nc.gpsimd.tensor_mul(at, at, nm_ap)
                                delay = 4
                            else:
                                dve_q.append((t + 1, "mul", (at, nm_ap)))
                                delay = 3
                        if g == NG - 1 and ks >= KS - 4:
                            # last group's tail: nothing behind these AVs on
                            # the PE, so drain them as early as possible
                            delay = min(delay, 2)
                        for qc in range(NQC):
                            av_q.append((t + delay, g, h, qg, ks, at, qc))
                        av_q.sort(key=lambda x: x[0])
                        dve_q.sort(key=lambda x: x[0])

                    # 3) due AV chunks, budget-limited for PE smoothness
                    budget = AV_BUDGET
                    while av_q and av_q[0][0] <= t and budget > 0:
                        _, pg, ph, pqg, pks, pat, pqc = av_q.pop(0)
                        budget -= 1
                        if emit_av_chunk(pg, ph, pqg, pks, pat, pqc):
                            dve_q.append((t + 1, "evac", pg))
                            dve_q.sort(key=lambda x: x[0])

            for _ in range(reps):
                emit_body()
    nc.compile()
    return nc


_CACHE = {}


def _get_nc():
    if "nc" not in _CACHE:
        _CACHE["nc"] = build_program()
    return _CACHE["nc"]


def make_in_maps(q, k, v, mask, s=S):
    """Shard full inputs into 8 per-core input maps (host-side layout prep)."""
    q = np.asarray(q, dtype=np.float32)
    k = np.asarray(k, dtype=np.float32)
    v = np.asarray(v, dtype=np.float32)
    mask = np.asarray(mask)
    nh = q.shape[-1] // HD
    ks_n = s // 128
    in_maps = []
    for c in range(NCORES):
        b, g = divmod(c, NCORES // B)
        h0 = HPC * g
        qs = q[b].reshape(s, nh, HD)[:, h0:h0 + HPC, :]      # [s, HPC, 64]
        ks_ = k[b].reshape(s, nh, HD)[:, h0:h0 + HPC, :]
        qkT = np.empty((2, HPC * HD, s), ml_dtypes.bfloat16)
        qkT[0] = qs.transpose(1, 2, 0).reshape(HPC * HD, s)
        qkT[1] = ks_.transpose(1, 2, 0).reshape(HPC * HD, s)
        # V' = [V | 1] laid out [128, KS, HPC, 65] fp16
        vh = v[b, :, h0 * HD:(h0 + HPC) * HD].reshape(ks_n, 128, HPC, HD)
        vp = np.ones((128, ks_n, HPC, HD + 1), np.float16)
        vp[:, :, :, :HD] = vh.transpose(1, 0, 2, 3)
        nmT = np.ascontiguousarray((~mask[b]).T).astype(np.float16)
        in_maps.append({"qkT": qkT, "vp": vp, "nmT": nmT})
    return in_maps


def assemble_out(results, s=S, d=D):
    """results[c]["out"]: [NG, 65, QG] bf16 (numerator rows 0:64, Z row 64).
    Normalize, transpose, and merge heads."""
    QG = 1024 if s >= 1024 else s
    NQG = s // QG
    out = np.empty((B, s, d), np.float32)
    for c in range(NCORES):
        b, gc = divmod(c, NCORES // B)
        h0 = HPC * gc
        r = np.asarray(results[c]["out"], dtype=np.float32)
        for g in range(HPC * NQG):
            h, qg = divmod(g, NQG)
            num = r[g, :HD, :]                # [64, QG]
            z = r[g, HD, :]                   # [QG]
            blk = (num / z).T                 # [QG, 64]
            col = (h0 + h) * HD
            out[b, qg * QG:(qg + 1) * QG, col:col + HD] = blk
    return out


def kernel(q, k, v, mask):
    nc = _get_nc()
    in_maps = make_in_maps(q, k, v, mask)
    res = run_bass_kernel_spmd(nc, in_maps, list(range(NCORES))).results
    return assemble_out(res)


# revision 55
# speedup vs baseline: 1.2050x; 1.0002x over previous
# Multi-head attention (B=2, S=2048, D=1024, H=16, head_dim=64) with bool mask,
# sharded across 8 TRN2 NeuronCores: core c -> batch c//4, heads 4*(c%4)..4*(c%4)+3.
#
# Per-core device kernel (scores computed transposed: scoresT[k, q]):
#   scoresT = K @ Q^T              (PE bf16, per 128-k strip, into PSUM)
#   at      = exp(scoresT/8)*nmT   split between two engines:
#              - ACT strips: ACT exp (scale=1/8) -> fp16, then DVE/Pool mask mul
#              - DVE strips: custom DVE op1 p=poly4(x)~e^(x/64) -> fp16,
#                custom DVE op2 at=(p^2)^2)^2*mask (mask fused; m^8=m)
#   psO     = [V|1]^T @ at         (PE fp16 accumulate over strips; row 64 = Z)
#   evac    = copy psO -> SBUF bf16 (DVE), DMA out (numerator+Z, unnormalized)
# Host side: shard inputs, pre-transpose Q/K per head (bf16), build V'=[V|1]
# (fp16), inverted-transposed mask (fp16); after run: out = (num/Z)^T merge.
#
# The custom DVE ops offload ~1/6 of the exp work from the ACT engine (the
# baseline bottleneck at ~133us busy) to the otherwise-idle DVE ALU pipeline,
# balancing PE/ACT/DVE at ~110us each.

import sys

import numpy as np

for _p in ("/opt/trn_rl_repo",):
    if _p not in sys.path:
        sys.path.insert(0, _p)

import ml_dtypes

import concourse.bass as bass  # noqa: F401  (engine types reachable via nc)
import concourse.tile as tile
from concourse import bacc, mybir
from concourse.bass_utils import run_bass_kernel_spmd

F32 = mybir.dt.float32
BF16 = mybir.dt.bfloat16
FP16 = mybir.dt.float16

S = 2048          # sequence length
HD = 64           # head dim
HPC = 4           # heads per core
NCORES = 8
B = 2
H = 16
D = H * HD

# e^(x/64) ~= 1 + c1 x + c2 x^2 + c3 x^3 + c4 x^4 on |x| <= 52 (~6.5 sigma of
# the N(0,64) score distribution); rel err 2.5e-4, x8 through the squarings
# -> ~2e-3 weight noise on DVE strips (output impact ~0.1%).
POLY_C = (1.5610800e-02, 1.2226440e-04, 6.6047774e-07, 2.4435055e-09)

_DVE_OPS = {}


def _register_dve_ops():
    """Register the two custom DVE ops (idempotent). Uses the documented
    extension point (dve_ops.OPS) at runtime since the repo is read-only."""
    if _DVE_OPS:
        return _DVE_OPS
    from concourse import dve_ops
    from concourse.dve_spec import (
        C0,
        C1,
        C2,
        C3,
        One,
        Spec,
        Src0,
        Src1,
        _has_src1,
        _spill_c3_to_src1,
        lower,
    )
    from concourse.dve_uop import DveOpSpec

    def _mk(name, body, reference, subdim=False):
        existing = {op.name: op for op in dve_ops.OPS}
        if name in existing:
            _DVE_OPS[name] = existing[name]
            return existing[name]
        spec = Spec(body=body, reference=reference)
        shas = {}
        for ver in ("v3", "v4"):
            try:
                uops = lower(spec, ver=ver)
                probe = DveOpSpec(
                    name=name, opcode=31, uops=uops, rd1_en=_has_src1(spec)
                )
                shas[ver] = probe.sha(ver)
            except Exception:
                pass
        op = dve_ops.DveOp(name, spec, subdim=subdim, uops_sha=shas)
        dve_ops.OPS.append(op)
        dve_ops.CUSTOM_DVE_SPECS[name] = spec
        dve_ops._SUB_OPCODE_FOR_NAME[name] = dve_ops._CUSTOM_DVE_ROW_BASE + (
            len(dve_ops.OPS) - 1
        )
        assert dve_ops._SUB_OPCODE_FOR_NAME[name] < 0x20
        _DVE_OPS[name] = op
        return op

    # op1: p = 1 + x(c1 + x(c2 + x(c3 + x c4)))   [8 ALU stages]
    # binding: s0=c4, s1=c3, imm2=c2, in1=[P,1] c1 (spilled C3)
    body1 = ((((Src0 * C0 + C1) * Src0 + C2) * Src0 + C3) * Src0) + One
    body1 = _spill_c3_to_src1(body1)

    def ref1(in0, in1, s0, s1, imm2):
        x = np.asarray(in0, np.float32)
        c1v = np.asarray(in1, np.float32)
        return ((((x * s0 + s1) * x + imm2) * x + c1v) * x) + np.float32(1.0)

    _mk("ANT_EXP64_P4", body1, ref1)

    # op2: at = (((p^2)^2)^2) * mask   [4 ALU stages]
    body2 = (Src0 * Src0)
    body2 = body2 * body2
    body2 = body2 * body2
    body2 = body2 * Src1

    def ref2(in0, in1, s0, s1, imm2):
        p = np.asarray(in0, np.float32)
        p8 = ((p * p) ** 2) ** 2
        return p8 * np.asarray(in1, np.float32)

    _mk("ANT_SQ3_MUL", body2, ref2)
    return _DVE_OPS


# Strips (by ks index) whose exp runs on the DVE via the custom ops, per
# group; and ACT strips whose mask-mul runs on Pool instead of DVE.
DVE_KS = (2, 6, 10)             # 3 per group x 8 groups = 24 DVE strips
POOL_MASK_KS = (0, 3, 7, 11)    # mask muls moved off DVE to the idle Pool


def build_program(s=S, reps=1, dve_ks=DVE_KS, pool_mask_ks=POOL_MASK_KS,
                  dve_ks0=(2, 6, 10), extra=(), n_psS=3, n_psO=1,
                  n_warm=20, av_budget=3, act_d=3, dve_d=5, pool_d=4):
    # extra: iterable of (group, ks) additionally offloaded to the DVE
    """Build the single-core SPMD program. Returns the compiled Bacc object."""
    ops = _register_dve_ops()
    op_poly = ops["ANT_EXP64_P4"]
    op_sq3 = ops["ANT_SQ3_MUL"]

    nc = bacc.Bacc()

    KS = s // 128            # number of k strips
    QG = 1024 if s >= 1024 else s   # q group width
    NQG = s // QG            # q groups
    NQC = max(QG // 512, 1)  # 512-wide matmul chunks per q group
    QC = min(512, QG)        # matmul chunk width
    NG = HPC * NQG           # groups per core

    c4, c3, c2, c1 = POLY_C[3], POLY_C[2], POLY_C[1], POLY_C[0]

    qkT_d = nc.declare_dram_parameter("qkT", [2, HPC * HD, s], BF16, isOutput=False)
    vp_d = nc.declare_dram_parameter("vp", [128, KS, HPC, HD + 1], FP16, isOutput=False)
    nmT_d = nc.declare_dram_parameter("nmT", [s, s], FP16, isOutput=False)
    out_d = nc.declare_dram_parameter("out", [NG, HD + 1, QG], BF16, isOutput=True)

    nm_view = nmT_d[:].rearrange("(ks p) q -> p ks q", p=128)

    with tile.TileContext(nc) as tc:
        with (
            tc.tile_pool(name="const", bufs=1) as const,
            tc.tile_pool(name="wq", bufs=1) as wq,
            tc.tile_pool(name="attn", bufs=20) as apool,
            tc.tile_pool(name="ppool", bufs=3) as ppool,
            tc.tile_pool(name="evac", bufs=2) as epool,
            tc.tile_pool(name="psS", bufs=n_psS, space="PSUM") as psS_pool,
            tc.tile_pool(name="psO", bufs=n_psO, space="PSUM") as psO_pool,
        ):
            # [P,1] scalar for the spilled 4th poly coefficient.
            c1t = const.tile([128, 1], F32)
            nc.vector.memset(c1t, c1)
            # [P,1] exp bias for the mask-folded tail strips
            nbias = const.tile([128, 1], F32)
            nc.vector.memset(nbias, -1250.0)

            # Preload the exp table while the first DMAs stream.
            warm = const.tile([128, 1], F32)
            nc.vector.memset(warm, 0.0)
            nc.scalar.activation(warm, warm, mybir.ActivationFunctionType.Exp)

            # Warm the PE HAM clock gate: ~3us of dummy matmuls so the first
            # real QKs run at full frequency.
            zb = const.tile([128, 128], BF16)
            nc.vector.memset(zb, 0.0)
            for _ in range(n_warm):
                wmm = psS_pool.tile([128, QG], F32, tag="psS")
                nc.tensor.matmul(
                    wmm[:, :128], lhsT=zb[0:64, :], rhs=zb[0:64, :],
                    start=True, stop=True,
                )

            # 10000 * identity (fp16): folds the mask into psS for the tail
            # strips via a PE copy-matmul, so their at needs no mask multiply
            from concourse.masks import make_identity

            identK = const.tile([128, 128], FP16)
            make_identity(nc, identK)
            nc.gpsimd.tensor_scalar_mul(identK, identK, 10000.0)

            def emit_body():
                # Input staging tiles
                qks = []
                for pair in range(HPC // 2):
                    qk = wq.tile([128, 2, s], BF16, tag=f"qkT{pair}")
                    qks.append(qk)
                vp_sb = wq.tile([128, KS, HPC, HD + 1], FP16, tag="vp")
                nm_sb = wq.tile([128, KS * s], FP16, tag="nm")

                def qk_src(pair):
                    return qkT_d[:, 128 * pair:128 * pair + 128, :].rearrange(
                        "t p s -> p t s"
                    )

                # DMA schedule: sync (SP HWDGE) carries the critical-path
                # early inputs; pool (SWDGE) carries late inputs + outputs.
                KH = KS // 2
                # k half of pair0 (cols 0:s) and q half cols of group 0.
                # Priority order for the serialized DMA-engine resource:
                # first QK operands for strips 0-3, then mask strips as the
                # consumer stream needs them, V' halves and the later q/k
                # windows just in time, second head pair and the qg1 mask
                # halves last.
                HQ = QG // 2
                nm_dst = lambda ks, a, b: nm_sb[:, ks * s + a:ks * s + b]

                def dq(dst, src):
                    nc.sync.dma_start(out=dst, in_=src)

                # just-in-time order for the serialized DMA-engine resource:
                # pair0 k/q + nm-a first (slots 0-31), pair1 next (slot 32+),
                # the qg1 q-columns and mask halves last (slot 64+)
                dq(qks[0][:, 1, 0:HQ], qk_src(0)[:, 1, 0:HQ])    # k strips 0-3
                dq(qks[0][:, 0, 0:HQ], qk_src(0)[:, 0, 0:HQ])    # q chunk 0
                dq(qks[0][:, 0, HQ:QG], qk_src(0)[:, 0, HQ:QG])  # q chunk 1
                dq(nm_dst(0, 0, QG), nm_view[:, 0, 0:QG])
                dq(qks[0][:, 1, HQ:QG], qk_src(0)[:, 1, HQ:QG])  # k strips 4-7
                dq(nm_dst(1, 0, QG), nm_view[:, 1, 0:QG])
                dq(vp_sb[:, :KH], vp_d[:, :KH])
                dq(nm_dst(2, 0, QG), nm_view[:, 2, 0:QG])
                dq(nm_dst(3, 0, QG), nm_view[:, 3, 0:QG])
                dq(nm_dst(4, 0, QG), nm_view[:, 4, 0:QG])
                if s > QG:
                    dq(qks[0][:, 1, QG:s], qk_src(0)[:, 1, QG:s])  # k 8-15
                dq(nm_dst(5, 0, QG), nm_view[:, 5, 0:QG])
                dq(nm_dst(6, 0, QG), nm_view[:, 6, 0:QG])
                dq(vp_sb[:, KH:], vp_d[:, KH:])
                for ks in range(7, KS):
                    dq(nm_dst(ks, 0, QG), nm_view[:, ks, 0:QG])
                for pair in range(1, HPC // 2):
                    dq(qks[pair][:, 1, :], qk_src(pair)[:, 1, :])
                    dq(qks[pair][:, 0, 0:QG], qk_src(pair)[:, 0, 0:QG])
                if s > QG:
                    dq(qks[0][:, 0, QG:s], qk_src(0)[:, 0, QG:s])
                    for pair in range(1, HPC // 2):
                        dq(qks[pair][:, 0, QG:s], qk_src(pair)[:, 0, QG:s])
                    for ks in range(KS):
                        dq(nm_dst(ks, QG, s), nm_view[:, ks, QG:s])

                # qg-major: all head groups sweep q-window 0 first, pushing
                # the qg1 mask-DMA deadline from ~slot 16 to ~slot 64
                groups = [(h, qg) for qg in range(NQG) for h in range(HPC)]
                NT = NG * KS  # total strips

                # per-group / scheduler state
                psO = {}            # g -> psum tile
                n_chunks = {}       # g -> AV chunks emitted
                av_q = []           # (min_slot, g, h, qg, ks, at, qc)
                dve_q = []          # (min_slot, kind, payload) for DVE stream
                AV_BUDGET = av_budget  # AV chunks per slot (> steady-state 2
                                       # so backlogs drain)

                ev_tiles = {}

                def emit_av_chunk(g, h, qg, ks, at, qc):
                    if not n_chunks.get(g):
                        psO[g] = psO_pool.tile(
                            [HD + 1, QG], F32, tag="psO", name=f"psO{g}"
                        )
                        n_chunks[g] = [0] * NQC
                    n_chunks[g][qc] += 1
                    nc.tensor.matmul(
                        psO[g][:, qc * QC:(qc + 1) * QC],
                        lhsT=vp_sb[:, ks, h, :],
                        rhs=at[:, qc * QC:(qc + 1) * QC],
                        start=n_chunks[g][qc] == 1,
                        stop=n_chunks[g][qc] == KS,
                    )
                    # column-range qc complete -> its evac can go
                    return n_chunks[g][qc] == KS

                def emit_evac(g, qc):
                    if g not in ev_tiles:
                        ev_tiles[g] = epool.tile(
                            [HD + 1, QG], BF16, tag="ev", name=f"ev{g}"
                        )
                    ev = ev_tiles[g]
                    sl = slice(qc * QC, (qc + 1) * QC)
                    nc.vector.tensor_copy(ev[:, sl], psO[g][:, sl])
                    eng = nc.scalar if (g == NG - 1 and qc == 1) else nc.sync
                    eng.dma_start(out=out_d[g][:, sl], in_=ev[:, sl])

                for t in range(NT + 8):
                    # 1) QK strip -> psS (its psS-slot dep resolves earliest)
                    if t < NT:
                        g, ks = divmod(t, KS)
                        h, qg = groups[g]
                        q0 = qg * QG
                        base = 64 * (h % 2)
                        qt_r = qks[h // 2][:, 0, :]
                        kt_r = qks[h // 2][:, 1, :]
                        # group 0's DVE strips sit later so the pipeline-fill
                        # mask backlog on the DVE has drained by then
                        is_dve = ks in (dve_ks0 if g == 0 else dve_ks) or (
                            (g, ks) in extra
                        )
                        # tail strips: fold the mask into psS via a PE
                        # copy-matmul (10000*nm) + exp bias so the at needs
                        # no separate mask multiply after the final exp
                        trick = g == NG - 1 and ks >= KS - 2

                        psS = psS_pool.tile([128, QG], F32, tag="psS")
                        if trick:
                            for qc in range(NQC):
                                nc.tensor.matmul(
                                    psS[:, qc * QC:(qc + 1) * QC],
                                    lhsT=identK[:, :],
                                    rhs=nm_sb[:, ks * s + q0 + qc * QC:
                                              ks * s + q0 + (qc + 1) * QC],
                                    start=True,
                                    stop=False,
                                    skip_group_check=True,
                                )
                        for qc in range(NQC):
                            nc.tensor.matmul(
                                psS[:, qc * QC:(qc + 1) * QC],
                                lhsT=kt_r[base:base + HD,
                                          ks * 128:(ks + 1) * 128],
                                rhs=qt_r[base:base + HD,
                                         q0 + qc * QC:q0 + (qc + 1) * QC],
                                start=not trick,
                                stop=True,
                                skip_group_check=trick,
                            )

                    # 2) deferred DVE-stream work due now: these became ready
                    # before this slot's poly (DVE wait queues are FIFO, so
                    # emission order must match readiness order)
                    while dve_q and dve_q[0][0] <= t:
                        _, kind, pl = dve_q.pop(0)
                        if kind == "mul":
                            at_, nm_ = pl
                            nc.vector.tensor_mul(at_, at_, nm_)
                        elif kind == "sq3":
                            at_, p_, nm_ = pl
                            nc.vector._custom_dve(
                                op_sq3, out=at_[:], in0=p_[:], in1=nm_,
                            )
                        else:  # evac
                            emit_evac(*pl)

                    if t < NT:
                        nm_ap = nm_sb[:, ks * s + q0:ks * s + q0 + QG]
                        at = apool.tile([128, QG], FP16, tag="at")
                        if is_dve:
                            p = ppool.tile([128, QG], FP16, tag="p")
                            nc.vector._custom_dve(
                                op_poly, out=p[:], in0=psS[:], in1=c1t[:],
                                s0=c4, s1=c3, imm2=c2,
                            )
                            dve_q.append((t + 2, "sq3", (at, p, nm_ap)))
                            delay = dve_d
                        elif trick:
                            nc.scalar.activation(
                                at, psS, mybir.ActivationFunctionType.Exp,
                                scale=0.125, bias=nbias[:],
                            )
                            delay = 1
                        elif t < 2:
                            # pipeline fill: exp per 512-half so the first
                            # activation starts as soon as QK chunk 0 lands
                            for qc in range(NQC):
                                sl = slice(qc * QC, (qc + 1) * QC)
                                nc.scalar.activation(
                                    at[:, sl], psS[:, sl],
                                    mybir.ActivationFunctionType.Exp,
                                    scale=0.125,
                                )
                            dve_q.append((t + 1, "mul", (at, nm_ap)))
                            delay = act_d
                        else:
                            nc.scalar.activation(
                                at, psS, mybir.ActivationFunctionType.Exp,
                                scale=0.125,
                            )
                            if ks in pool_mask_ks:
                                nc.gpsimd.tensor_mul(at, at, nm_ap)
                                delay = pool_d
                            else:
                                dve_q.append((t + 1, "mul", (at, nm_ap)))
                                delay = act_d
                        if g == NG - 1 and ks >= KS - 4:
                            # last group's tail: nothing behind these AVs on
                            # the PE, so drain them as early as possible
                            delay = min(delay, 2) if not trick else 1
                        for qc in range(NQC):
                            av_q.append((t + delay, g, h, qg, ks, at, qc))
                        av_q.sort(key=lambda x: x[0])
                        dve_q.sort(key=lambda x: x[0])

                    # 3) due AV chunks, budget-limited for PE smoothness
                    budget = AV_BUDGET if t < NT else 8
                    while av_q and av_q[0][0] <= t and budget > 0:
                        _, pg, ph, pqg, pks, pat, pqc = av_q.pop(0)
                        budget -= 1
                        if emit_av_chunk(pg, ph, pqg, pks, pat, pqc):
                            dve_q.append((t + 1, "evac", (pg, pqc)))
                            dve_q.sort(key=lambda x: x[0])

            for _ in range(reps):
                emit_body()
    nc.compile()
    return nc


_CACHE = {}


def _get_nc():
    if "nc" not in _CACHE:
        _CACHE["nc"] = build_program()
    return _CACHE["nc"]


def make_in_maps(q, k, v, mask, s=S):
    """Shard full inputs into 8 per-core input maps (host-side layout prep)."""
    q = np.asarray(q, dtype=np.float32)
    k = np.asarray(k, dtype=np.float32)
    v = np.asarray(v, dtype=np.float32)
    mask = np.asarray(mask)
    nh = q.shape[-1] // HD
    ks_n = s // 128
    in_maps = []
    for c in range(NCORES):
        b, g = divmod(c, NCORES // B)
        h0 = HPC * g
        qs = q[b].reshape(s, nh, HD)[:, h0:h0 + HPC, :]      # [s, HPC, 64]
        ks_ = k[b].reshape(s, nh, HD)[:, h0:h0 + HPC, :]
        qkT = np.empty((2, HPC * HD, s), ml_dtypes.bfloat16)
        qkT[0] = qs.transpose(1, 2, 0).reshape(HPC * HD, s)
        qkT[1] = ks_.transpose(1, 2, 0).reshape(HPC * HD, s)
        # V' = [V | 1] laid out [128, KS, HPC, 65] fp16
        vh = v[b, :, h0 * HD:(h0 + HPC) * HD].reshape(ks_n, 128, HPC, HD)
        vp = np.ones((128, ks_n, HPC, HD + 1), np.float16)
        vp[:, :, :, :HD] = vh.transpose(1, 0, 2, 3)
        nmT = np.ascontiguousarray((~mask[b]).T).astype(np.float16)
        in_maps.append({"qkT": qkT, "vp": vp, "nmT": nmT})
    return in_maps


def assemble_out(results, s=S, d=D):
    """results[c]["out"]: [NG, 65, QG] bf16 (numerator rows 0:64, Z row 64).
    Normalize, transpose, and merge heads."""
    QG = 1024 if s >= 1024 else s
    NQG = s // QG
    out = np.empty((B, s, d), np.float32)
    for c in range(NCORES):
        b, gc = divmod(c, NCORES // B)
        h0 = HPC * gc
        r = np.asarray(results[c]["out"], dtype=np.float32)
        grps = [(h, qg) for qg in range(NQG) for h in range(HPC)]
        for g, (h, qg) in enumerate(grps):
            num = r[g, :HD, :]                # [64, QG]
            z = r[g, HD, :]                   # [QG]
            blk = (num / z).T                 # [QG, 64]
            col = (h0 + h) * HD
            out[b, qg * QG:(qg + 1) * QG, col:col + HD] = blk
    return out


def kernel(q, k, v, mask):
    nc = _get_nc()
    in_maps = make_in_maps(q, k, v, mask)
    res = run_bass_kernel_spmd(nc, in_maps, list(range(NCORES))).results
    return assemble_out(res)


# revision 57
# speedup vs baseline: 1.2153x; 1.0085x over previous
# Multi-head attention (B=2, S=2048, D=1024, H=16, head_dim=64) with bool mask,
# sharded across 8 TRN2 NeuronCores: core c -> batch c//4, heads 4*(c%4)..4*(c%4)+3.
#
# Per-core device kernel (scores computed transposed: scoresT[k, q]):
#   scoresT = K @ Q^T              (PE bf16, per 128-k strip, into PSUM)
#   at      = exp(scoresT/8)*nmT   split between two engines:
#              - ACT strips: ACT exp (scale=1/8) -> fp16, then DVE/Pool mask mul
#              - DVE strips: custom DVE op1 p=poly4(x)~e^(x/64) -> fp16,
#                custom DVE op2 at=(p^2)^2)^2*mask (mask fused; m^8=m)
#   psO     = [V|1]^T @ at         (PE fp16 accumulate over strips; row 64 = Z)
#   evac    = copy psO -> SBUF bf16 (DVE), DMA out (numerator+Z, unnormalized)
# Host side: shard inputs, pre-transpose Q/K per head (bf16), build V'=[V|1]
# (fp16), inverted-transposed mask (fp16); after run: out = (num/Z)^T merge.
#
# The custom DVE ops offload ~1/6 of the exp work from the ACT engine (the
# baseline bottleneck at ~133us busy) to the otherwise-idle DVE ALU pipeline,
# balancing PE/ACT/DVE at ~110us each.

import sys

import numpy as np

for _p in ("/opt/trn_rl_repo",):
    if _p not in sys.path:
        sys.path.insert(0, _p)

import ml_dtypes

import concourse.bass as bass  # noqa: F401  (engine types reachable via nc)
import concourse.tile as tile
from concourse import bacc, mybir
from concourse.bass_utils import run_bass_kernel_spmd

F32 = mybir.dt.float32
BF16 = mybir.dt.bfloat16
FP16 = mybir.dt.float16

S = 2048          # sequence length
HD = 64           # head dim
HPC = 4           # heads per core
NCORES = 8
B = 2
H = 16
D = H * HD

# e^(x/64) ~= 1 + c1 x + c2 x^2 + c3 x^3 + c4 x^4 on |x| <= 52 (~6.5 sigma of
# the N(0,64) score distribution); rel err 2.5e-4, x8 through the squarings
# -> ~2e-3 weight noise on DVE strips (output impact ~0.1%).
POLY_C = (1.5610800e-02, 1.2226440e-04, 6.6047774e-07, 2.4435055e-09)

_DVE_OPS = {}


def _register_dve_ops():
    """Register the two custom DVE ops (idempotent). Uses the documented
    extension point (dve_ops.OPS) at runtime since the repo is read-only."""
    if _DVE_OPS:
        return _DVE_OPS
    from concourse import dve_ops
    from concourse.dve_spec import (
        C0,
        C1,
        C2,
        C3,
        One,
        Spec,
        Src0,
        Src1,
        _has_src1,
        _spill_c3_to_src1,
        lower,
    )
    from concourse.dve_uop import DveOpSpec

    def _mk(name, body, reference, subdim=False):
        existing = {op.name: op for op in dve_ops.OPS}
        if name in existing:
            _DVE_OPS[name] = existing[name]
            return existing[name]
        spec = Spec(body=body, reference=reference)
        shas = {}
        for ver in ("v3", "v4"):
            try:
                uops = lower(spec, ver=ver)
                probe = DveOpSpec(
                    name=name, opcode=31, uops=uops, rd1_en=_has_src1(spec)
                )
                shas[ver] = probe.sha(ver)
            except Exception:
                pass
        op = dve_ops.DveOp(name, spec, subdim=subdim, uops_sha=shas)
        dve_ops.OPS.append(op)
        dve_ops.CUSTOM_DVE_SPECS[name] = spec
        dve_ops._SUB_OPCODE_FOR_NAME[name] = dve_ops._CUSTOM_DVE_ROW_BASE + (
            len(dve_ops.OPS) - 1
        )
        assert dve_ops._SUB_OPCODE_FOR_NAME[name] < 0x20
        _DVE_OPS[name] = op
        return op

    # op1: p = 1 + x(c1 + x(c2 + x(c3 + x c4)))   [8 ALU stages]
    # binding: s0=c4, s1=c3, imm2=c2, in1=[P,1] c1 (spilled C3)
    body1 = ((((Src0 * C0 + C1) * Src0 + C2) * Src0 + C3) * Src0) + One
    body1 = _spill_c3_to_src1(body1)

    def ref1(in0, in1, s0, s1, imm2):
        x = np.asarray(in0, np.float32)
        c1v = np.asarray(in1, np.float32)
        return ((((x * s0 + s1) * x + imm2) * x + c1v) * x) + np.float32(1.0)

    _mk("ANT_EXP64_P4", body1, ref1)

    # op2: at = (((p^2)^2)^2) * mask   [4 ALU stages]
    body2 = (Src0 * Src0)
    body2 = body2 * body2
    body2 = body2 * body2
    body2 = body2 * Src1

    def ref2(in0, in1, s0, s1, imm2):
        p = np.asarray(in0, np.float32)
        p8 = ((p * p) ** 2) ** 2
        return p8 * np.asarray(in1, np.float32)

    _mk("ANT_SQ3_MUL", body2, ref2)
    return _DVE_OPS


# Strips (by ks index) whose exp runs on the DVE via the custom ops, per
# group; and ACT strips whose mask-mul runs on Pool instead of DVE.
DVE_KS = (2, 6, 10)             # 3 per group x 8 groups = 24 DVE strips
POOL_MASK_KS = (0, 3, 7, 11)    # mask muls moved off DVE to the idle Pool


def build_program(s=S, reps=1, dve_ks=DVE_KS, pool_mask_ks=POOL_MASK_KS,
                  dve_ks0=(2, 6, 10), extra=(), n_psS=3, n_psO=1,
                  n_warm=20, av_budget=3, act_d=3, dve_d=5, pool_d=4,
                  evac_single=True, last_d=2, apool_bufs=20):
    # extra: iterable of (group, ks) additionally offloaded to the DVE
    """Build the single-core SPMD program. Returns the compiled Bacc object."""
    ops = _register_dve_ops()
    op_poly = ops["ANT_EXP64_P4"]
    op_sq3 = ops["ANT_SQ3_MUL"]

    nc = bacc.Bacc()

    KS = s // 128            # number of k strips
    QG = 1024 if s >= 1024 else s   # q group width
    NQG = s // QG            # q groups
    NQC = max(QG // 512, 1)  # 512-wide matmul chunks per q group
    QC = min(512, QG)        # matmul chunk width
    NG = HPC * NQG           # groups per core

    c4, c3, c2, c1 = POLY_C[3], POLY_C[2], POLY_C[1], POLY_C[0]

    qkT_d = nc.declare_dram_parameter("qkT", [2, HPC * HD, s], BF16, isOutput=False)
    vp_d = nc.declare_dram_parameter("vp", [128, KS, HPC, HD + 1], FP16, isOutput=False)
    nmT_d = nc.declare_dram_parameter("nmT", [s, s], FP16, isOutput=False)
    out_d = nc.declare_dram_parameter("out", [NG, HD + 1, QG], BF16, isOutput=True)

    nm_view = nmT_d[:].rearrange("(ks p) q -> p ks q", p=128)

    with tile.TileContext(nc) as tc:
        with (
            tc.tile_pool(name="const", bufs=1) as const,
            tc.tile_pool(name="wq", bufs=1) as wq,
            tc.tile_pool(name="attn", bufs=apool_bufs) as apool,
            tc.tile_pool(name="ppool", bufs=3) as ppool,
            tc.tile_pool(name="evac", bufs=2) as epool,
            tc.tile_pool(name="psS", bufs=n_psS, space="PSUM") as psS_pool,
            tc.tile_pool(name="psO", bufs=n_psO, space="PSUM") as psO_pool,
        ):
            # [P,1] scalar for the spilled 4th poly coefficient.
            c1t = const.tile([128, 1], F32)
            nc.vector.memset(c1t, c1)
            # [P,1] exp bias for the mask-folded tail strips
            nbias = const.tile([128, 1], F32)
            nc.vector.memset(nbias, -1250.0)

            # Preload the exp table while the first DMAs stream.
            warm = const.tile([128, 1], F32)
            nc.vector.memset(warm, 0.0)
            nc.scalar.activation(warm, warm, mybir.ActivationFunctionType.Exp)

            # Warm the PE HAM clock gate: ~3us of dummy matmuls so the first
            # real QKs run at full frequency.
            zb = const.tile([128, 128], BF16)
            nc.vector.memset(zb, 0.0)
            for _ in range(n_warm):
                wmm = psS_pool.tile([128, QG], F32, tag="psS")
                nc.tensor.matmul(
                    wmm[:, :128], lhsT=zb[0:64, :], rhs=zb[0:64, :],
                    start=True, stop=True,
                )

            # 10000 * identity (fp16): folds the mask into psS for the tail
            # strips via a PE copy-matmul, so their at needs no mask multiply
            from concourse.masks import make_identity

            identK = const.tile([128, 128], FP16)
            make_identity(nc, identK)
            nc.gpsimd.tensor_scalar_mul(identK, identK, 10000.0)

            def emit_body():
                # Input staging tiles
                qks = []
                for pair in range(HPC // 2):
                    qk = wq.tile([128, 2, s], BF16, tag=f"qkT{pair}")
                    qks.append(qk)
                vp_sb = wq.tile([128, KS, HPC, HD + 1], FP16, tag="vp")
                nm_sb = wq.tile([128, KS * s], FP16, tag="nm")

                def qk_src(pair):
                    return qkT_d[:, 128 * pair:128 * pair + 128, :].rearrange(
                        "t p s -> p t s"
                    )

                # DMA schedule: sync (SP HWDGE) carries the critical-path
                # early inputs; pool (SWDGE) carries late inputs + outputs.
                KH = KS // 2
                # k half of pair0 (cols 0:s) and q half cols of group 0.
                # Priority order for the serialized DMA-engine resource:
                # first QK operands for strips 0-3, then mask strips as the
                # consumer stream needs them, V' halves and the later q/k
                # windows just in time, second head pair and the qg1 mask
                # halves last.
                HQ = QG // 2
                nm_dst = lambda ks, a, b: nm_sb[:, ks * s + a:ks * s + b]

                def dq(dst, src):
                    nc.sync.dma_start(out=dst, in_=src)

                # just-in-time order for the serialized DMA-engine resource:
                # pair0 k/q + nm-a first (slots 0-31), pair1 next (slot 32+),
                # the qg1 q-columns and mask halves last (slot 64+)
                dq(qks[0][:, 1, 0:HQ], qk_src(0)[:, 1, 0:HQ])    # k strips 0-3
                dq(qks[0][:, 0, 0:HQ], qk_src(0)[:, 0, 0:HQ])    # q chunk 0
                dq(qks[0][:, 0, HQ:QG], qk_src(0)[:, 0, HQ:QG])  # q chunk 1
                dq(nm_dst(0, 0, QG), nm_view[:, 0, 0:QG])
                dq(qks[0][:, 1, HQ:QG], qk_src(0)[:, 1, HQ:QG])  # k strips 4-7
                dq(nm_dst(1, 0, QG), nm_view[:, 1, 0:QG])
                dq(vp_sb[:, :KH], vp_d[:, :KH])
                dq(nm_dst(2, 0, QG), nm_view[:, 2, 0:QG])
                dq(nm_dst(3, 0, QG), nm_view[:, 3, 0:QG])
                dq(nm_dst(4, 0, QG), nm_view[:, 4, 0:QG])
                if s > QG:
                    dq(qks[0][:, 1, QG:s], qk_src(0)[:, 1, QG:s])  # k 8-15
                dq(nm_dst(5, 0, QG), nm_view[:, 5, 0:QG])
                dq(nm_dst(6, 0, QG), nm_view[:, 6, 0:QG])
                dq(vp_sb[:, KH:], vp_d[:, KH:])
                for ks in range(7, KS):
                    dq(nm_dst(ks, 0, QG), nm_view[:, ks, 0:QG])
                for pair in range(1, HPC // 2):
                    dq(qks[pair][:, 1, :], qk_src(pair)[:, 1, :])
                    dq(qks[pair][:, 0, 0:QG], qk_src(pair)[:, 0, 0:QG])
                if s > QG:
                    dq(qks[0][:, 0, QG:s], qk_src(0)[:, 0, QG:s])
                    for pair in range(1, HPC // 2):
                        dq(qks[pair][:, 0, QG:s], qk_src(pair)[:, 0, QG:s])
                    for ks in range(KS):
                        dq(nm_dst(ks, QG, s), nm_view[:, ks, QG:s])

                # qg-major: all head groups sweep q-window 0 first, pushing
                # the qg1 mask-DMA deadline from ~slot 16 to ~slot 64
                groups = [(h, qg) for qg in range(NQG) for h in range(HPC)]
                NT = NG * KS  # total strips

                # per-group / scheduler state
                psO = {}            # g -> psum tile
                n_chunks = {}       # g -> AV chunks emitted
                av_q = []           # (min_slot, g, h, qg, ks, at, qc)
                dve_q = []          # (min_slot, kind, payload) for DVE stream
                AV_BUDGET = av_budget  # AV chunks per slot (> steady-state 2
                                       # so backlogs drain)

                ev_tiles = {}

                def emit_av_chunk(g, h, qg, ks, at, qc):
                    if not n_chunks.get(g):
                        psO[g] = psO_pool.tile(
                            [HD + 1, QG], F32, tag="psO", name=f"psO{g}"
                        )
                        n_chunks[g] = [0] * NQC
                    n_chunks[g][qc] += 1
                    nc.tensor.matmul(
                        psO[g][:, qc * QC:(qc + 1) * QC],
                        lhsT=vp_sb[:, ks, h, :],
                        rhs=at[:, qc * QC:(qc + 1) * QC],
                        start=n_chunks[g][qc] == 1,
                        stop=n_chunks[g][qc] == KS,
                    )
                    # column-range qc complete -> its evac can go
                    return n_chunks[g][qc] == KS

                def emit_evac(g, qc):
                    if g not in ev_tiles:
                        ev_tiles[g] = epool.tile(
                            [HD + 1, QG], BF16, tag="ev", name=f"ev{g}"
                        )
                    ev = ev_tiles[g]
                    if evac_single:
                        if qc == 0:
                            return
                        nc.vector.tensor_copy(ev, psO[g])
                        nc.sync.dma_start(out=out_d[g], in_=ev[:])
                        return
                    sl = slice(qc * QC, (qc + 1) * QC)
                    nc.vector.tensor_copy(ev[:, sl], psO[g][:, sl])
                    eng = nc.scalar if (g == NG - 1 and qc == 1) else nc.sync
                    eng.dma_start(out=out_d[g][:, sl], in_=ev[:, sl])

                for t in range(NT + 8):
                    # 1) QK strip -> psS (its psS-slot dep resolves earliest)
                    if t < NT:
                        g, ks = divmod(t, KS)
                        h, qg = groups[g]
                        q0 = qg * QG
                        base = 64 * (h % 2)
                        qt_r = qks[h // 2][:, 0, :]
                        kt_r = qks[h // 2][:, 1, :]
                        # group 0's DVE strips sit later so the pipeline-fill
                        # mask backlog on the DVE has drained by then
                        is_dve = ks in (dve_ks0 if g == 0 else dve_ks) or (
                            (g, ks) in extra
                        )
                        # tail strips: fold the mask into psS via a PE
                        # copy-matmul (10000*nm) + exp bias so the at needs
                        # no separate mask multiply after the final exp
                        trick = g == NG - 1 and ks >= KS - 2

                        psS = psS_pool.tile([128, QG], F32, tag="psS")
                        if trick:
                            for qc in range(NQC):
                                nc.tensor.matmul(
                                    psS[:, qc * QC:(qc + 1) * QC],
                                    lhsT=identK[:, :],
                                    rhs=nm_sb[:, ks * s + q0 + qc * QC:
                                              ks * s + q0 + (qc + 1) * QC],
                                    start=True,
                                    stop=False,
                                    skip_group_check=True,
                                )
                        for qc in range(NQC):
                            nc.tensor.matmul(
                                psS[:, qc * QC:(qc + 1) * QC],
                                lhsT=kt_r[base:base + HD,
                                          ks * 128:(ks + 1) * 128],
                                rhs=qt_r[base:base + HD,
                                         q0 + qc * QC:q0 + (qc + 1) * QC],
                                start=not trick,
                                stop=True,
                                skip_group_check=trick,
                            )

                    # 2) deferred DVE-stream work due now: these became ready
                    # before this slot's poly (DVE wait queues are FIFO, so
                    # emission order must match readiness order)
                    while dve_q and dve_q[0][0] <= t:
                        _, kind, pl = dve_q.pop(0)
                        if kind == "mul":
                            at_, nm_ = pl
                            nc.vector.tensor_mul(at_, at_, nm_)
                        elif kind == "sq3":
                            at_, p_, nm_ = pl
                            nc.vector._custom_dve(
                                op_sq3, out=at_[:], in0=p_[:], in1=nm_,
                            )
                        else:  # evac
                            emit_evac(*pl)

                    if t < NT:
                        nm_ap = nm_sb[:, ks * s + q0:ks * s + q0 + QG]
                        at = apool.tile([128, QG], FP16, tag="at")
                        if is_dve:
                            p = ppool.tile([128, QG], FP16, tag="p")
                            nc.vector._custom_dve(
                                op_poly, out=p[:], in0=psS[:], in1=c1t[:],
                                s0=c4, s1=c3, imm2=c2,
                            )
                            dve_q.append((t + 2, "sq3", (at, p, nm_ap)))
                            delay = dve_d
                        elif trick:
                            nc.scalar.activation(
                                at, psS, mybir.ActivationFunctionType.Exp,
                                scale=0.125, bias=nbias[:],
                            )
                            delay = 1
                        elif t < 2:
                            # pipeline fill: exp per 512-half so the first
                            # activation starts as soon as QK chunk 0 lands
                            for qc in range(NQC):
                                sl = slice(qc * QC, (qc + 1) * QC)
                                nc.scalar.activation(
                                    at[:, sl], psS[:, sl],
                                    mybir.ActivationFunctionType.Exp,
                                    scale=0.125,
                                )
                            dve_q.append((t + 1, "mul", (at, nm_ap)))
                            delay = act_d
                        else:
                            nc.scalar.activation(
                                at, psS, mybir.ActivationFunctionType.Exp,
                                scale=0.125,
                            )
                            if ks in pool_mask_ks:
                                nc.gpsimd.tensor_mul(at, at, nm_ap)
                                delay = pool_d
                            else:
                                dve_q.append((t + 1, "mul", (at, nm_ap)))
                                delay = act_d
                        if g == NG - 1 and ks >= KS - 4:
                            # last group's tail: nothing behind these AVs on
                            # the PE, so drain them as early as possible
                            delay = min(delay, last_d) if not trick else 1
                        for qc in range(NQC):
                            av_q.append((t + delay, g, h, qg, ks, at, qc))
                        av_q.sort(key=lambda x: x[0])
                        dve_q.sort(key=lambda x: x[0])

                    # 3) due AV chunks, budget-limited for PE smoothness
                    budget = AV_BUDGET if t < NT else 8
                    while av_q and av_q[0][0] <= t and budget > 0:
                        _, pg, ph, pqg, pks, pat, pqc = av_q.pop(0)
                        budget -= 1
                        if emit_av_chunk(pg, ph, pqg, pks, pat, pqc):
                            dve_q.append((t + 1, "evac", (pg, pqc)))
                            dve_q.sort(key=lambda x: x[0])

            for _ in range(reps):
                emit_body()
    nc.compile()
    return nc


_CACHE = {}


def _get_nc():
    if "nc" not in _CACHE:
        _CACHE["nc"] = build_program()
    return _CACHE["nc"]


def make_in_maps(q, k, v, mask, s=S):
    """Shard full inputs into 8 per-core input maps (host-side layout prep)."""
    q = np.asarray(q, dtype=np.float32)
    k = np.asarray(k, dtype=np.float32)
    v = np.asarray(v, dtype=np.float32)
    mask = np.asarray(mask)
    nh = q.shape[-1] // HD
    ks_n = s // 128
    in_maps = []
    for c in range(NCORES):
        b, g = divmod(c, NCORES // B)
        h0 = HPC * g
        qs = q[b].reshape(s, nh, HD)[:, h0:h0 + HPC, :]      # [s, HPC, 64]
        ks_ = k[b].reshape(s, nh, HD)[:, h0:h0 + HPC, :]
        qkT = np.empty((2, HPC * HD, s), ml_dtypes.bfloat16)
        qkT[0] = qs.transpose(1, 2, 0).reshape(HPC * HD, s)
        qkT[1] = ks_.transpose(1, 2, 0).reshape(HPC * HD, s)
        # V' = [V | 1] laid out [128, KS, HPC, 65] fp16
        vh = v[b, :, h0 * HD:(h0 + HPC) * HD].reshape(ks_n, 128, HPC, HD)
        vp = np.ones((128, ks_n, HPC, HD + 1), np.float16)
        vp[:, :, :, :HD] = vh.transpose(1, 0, 2, 3)
        nmT = np.ascontiguousarray((~mask[b]).T).astype(np.float16)
        in_maps.append({"qkT": qkT, "vp": vp, "nmT": nmT})
    return in_maps


def assemble_out(results, s=S, d=D):
    """results[c]["out"]: [NG, 65, QG] bf16 (numerator rows 0:64, Z row 64).
    Normalize, transpose, and merge heads."""
    QG = 1024 if s >= 1024 else s
    NQG = s // QG
    out = np.empty((B, s, d), np.float32)
    for c in range(NCORES):
        b, gc = divmod(c, NCORES // B)
        h0 = HPC * gc
        r = np.asarray(results[c]["out"], dtype=np.float32)
        grps = [(h, qg) for qg in range(NQG) for h in range(HPC)]
        for g, (h, qg) in enumerate(grps):
            num = r[g, :HD, :]                # [64, QG]
            z = r[g, HD, :]                   # [QG]
            blk = (num / z).T                 # [QG, 64]
            col = (h0 + h) * HD
            out[b, qg * QG:(qg + 1) * QG, col:col + HD] = blk
    return out


def kernel(q, k, v, mask):
    nc = _get_nc()
    in_maps = make_in_maps(q, k, v, mask)
    res = run_bass_kernel_spmd(nc, in_maps, list(range(NCORES))).results
    return assemble_out(res)
